# revision 6
# baseline (speedup 1.0000x reference)
"""Trainium2 Bass kernel for bilinear causal attention (no softmax).

Math (from the reference):
  Omega[b,h,t,u] = r_t^T Q^h r_u            (scores)
  out[b,t,:]     = sum_h sum_{u<=t} Omega[b,h,t,u] * (E^h r_u)

Shapes: r_prime [1,4,2048,256] f32, Q [1,8,256,256], E [1,8,256,256],
out [1,4,2048,256] f32.

Sharding over 8 NeuronCores: core = 2*b + hg handles batch b (4 batches)
and head-group hg (heads hg*4 .. hg*4+3).  Each core produces a partial
output summed over its 4 heads; the host adds the two head-group
partials per batch.  No on-chip collectives.

Per-core algorithm (all matmuls, bf16 inputs with f32 PSUM accumulation):
  KT[h]  :  KT[i',t] = sum_i Q[i,i'] rT[i,t]            ([256,2048])
  V[h]   :  V[u,i']  = sum_j rT[j,u] ET[j,i']           ([2048,256])
  ST     :  ST[u,t]  = sum_{i'} rT[i',u] KT[i',t]       (= Omega[t,u])
  OT     :  OT[i',t] += sum_h sum_u V[u,i'] ST_masked[u,t]

All matmuls run with a 512-wide moving operand (one f32 PSUM bank).
Causality at tile granularity: u-tiles above the t-chunk are skipped;
on the chunk diagonal only the valid column range [pl*128:512] is
computed and the triangular boundary is masked elementwise.  The output
is produced transposed ([i',t]) and transposed back on the host.
"""

import numpy as np
import ml_dtypes

N_T = 2048           # sequence length t
N_IN = 256           # feature dim (i, j, i' all 256)
CHUNK = 512          # matmul free-dim chunk (one PSUM bank of f32)
NCHUNKS = N_T // CHUNK          # 4
TPC = CHUNK // 128              # 4 tiles of 128 per chunk
TQ = N_T // 128                 # 16 row tiles
N_CORES = 8

_CACHE = {}


def _build_nc():
    from concourse import mybir, bacc, tile

    BF16 = mybir.dt.bfloat16
    F32 = mybir.dt.float32

    nc = bacc.Bacc(
        "TRN2", target_bir_lowering=False, debug=False, num_devices=N_CORES
    )
    # All inputs partition-major so each loads with few large DMAs.
    rt_d = nc.dram_tensor("rt", [128, 2, N_T], BF16, kind="ExternalInput").ap()
    q_d = nc.dram_tensor("q", [128, 4, 2, N_IN], BF16, kind="ExternalInput").ap()
    # et pairs two heads side by side: [p, head-pair, j-chunk, 512]
    et_d = nc.dram_tensor("et", [128, 2, 2, CHUNK], BF16, kind="ExternalInput").ap()
    mask_d = nc.dram_tensor(
        "cmask", [128, TPC * CHUNK], F32, kind="ExternalInput"
    ).ap()
    # transposed output: [i' chunk, i' in chunk, t]
    out_d = nc.dram_tensor("out", [2, 128, N_T], F32, kind="ExternalOutput").ap()

    with tile.TileContext(nc) as tc:
        with (
            tc.tile_pool(name="consts", bufs=1) as consts,
            tc.tile_pool(name="stsb", bufs=8) as stp,
            tc.tile_pool(name="outsb", bufs=3) as outp,
            tc.tile_pool(name="psum", bufs=1, space="PSUM") as psum,
        ):
            rt_sb = consts.tile([128, 2, N_T], BF16)
            q_sb = consts.tile([128, 4, 2, N_IN], BF16)
            et_sb = consts.tile([128, 2, 2, CHUNK], BF16)
            mask_sb = consts.tile([128, TPC * CHUNK], F32)
            kt_sb = consts.tile([128, 4, 2, N_T], BF16)
            # [p, head-pair, u-tile, (head-in-pair x i')]
            v_sb = consts.tile([128, 2, TQ, 2 * N_IN], BF16)

            # Inputs: q first (needed by the first KT matmuls), rt split
            # per 512-chunk so compute starts as soon as the first chunk
            # lands; et/mask go down the gpsimd queue in parallel.
            nc.sync.dma_start(out=q_sb[:], in_=q_d[:])
            for tcn in range(NCHUNKS):
                nc.sync.dma_start(
                    out=rt_sb[:, :, tcn * CHUNK : (tcn + 1) * CHUNK],
                    in_=rt_d[:, :, tcn * CHUNK : (tcn + 1) * CHUNK],
                )
            nc.gpsimd.dma_start(out=et_sb[:], in_=et_d[:])
            nc.gpsimd.dma_start(out=mask_sb[:], in_=mask_d[:])

            # Phase A: per head, KT = (rQ)^T; per head-pair, V = r @ E^T
            # (two heads share the 512-wide moving operand). Kept in SBUF
            # as bf16.
            for h in range(4):
                for ipc in range(2):
                    for tcn in range(NCHUNKS):
                        kt_ps = psum.tile([128, CHUNK], F32, tag="ps512", bufs=3)
                        for ic in range(2):
                            nc.tensor.matmul(
                                kt_ps[:],
                                q_sb[:, h, ic, ipc * 128 : (ipc + 1) * 128],
                                rt_sb[:, ic, tcn * CHUNK : (tcn + 1) * CHUNK],
                                start=(ic == 0),
                                stop=(ic == 1),
                            )
                        nc.any.tensor_copy(
                            kt_sb[:, h, ipc, tcn * CHUNK : (tcn + 1) * CHUNK],
                            kt_ps[:],
                        )
                if h % 2 == 0:
                    hp = h // 2
                    for ut in range(TQ):
                        v_ps = psum.tile([128, CHUNK], F32, tag="ps512", bufs=3)
                        for ic in range(2):
                            nc.tensor.matmul(
                                v_ps[:],
                                rt_sb[:, ic, ut * 128 : (ut + 1) * 128],
                                et_sb[:, hp, ic, :],
                                start=(ic == 0),
                                stop=(ic == 1),
                            )
                        nc.any.tensor_copy(v_sb[:, hp, ut, :], v_ps[:])

            # Phase B: per 512-wide t-chunk, stream ST tiles and accumulate
            # the transposed output in PSUM across heads and u-tiles.
            for c in range(NCHUNKS):
                ot_ps = [
                    psum.tile(
                        [128, CHUNK], F32, tag="ot", bufs=4, name=f"ot_{c}_{ipc}"
                    )
                    for ipc in range(2)
                ]
                for ut in range(TPC * c + TPC):
                    pl = ut - TPC * c  # >= 0 on the chunk diagonal
                    off = max(pl, 0) * 128
                    for h in range(4):
                        st_ps = psum.tile([128, CHUNK], F32, tag="ps512", bufs=3)
                        for ic in range(2):
                            nc.tensor.matmul(
                                st_ps[:, off:],
                                rt_sb[:, ic, ut * 128 : (ut + 1) * 128],
                                kt_sb[:, h, ic, c * CHUNK + off : (c + 1) * CHUNK],
                                start=(ic == 0),
                                stop=(ic == 1),
                            )
                        st_sb = stp.tile([128, CHUNK], BF16, tag="st")
                        if pl >= 0:
                            nc.vector.tensor_mul(
                                st_sb[:, off:],
                                st_ps[:, off:],
                                mask_sb[:, pl * CHUNK + off : (pl + 1) * CHUNK],
                            )
                        else:
                            nc.any.tensor_copy(st_sb[:], st_ps[:])
                        for ipc in range(2):
                            voff = (h % 2) * N_IN + ipc * 128
                            nc.tensor.matmul(
                                ot_ps[ipc][:, off:],
                                v_sb[:, h // 2, ut, voff : voff + 128],
                                st_sb[:, off:],
                                start=(ut == 0 and h == 0),
                                stop=(ut == TPC * c + TPC - 1 and h == 3),
                            )
                for ipc in range(2):
                    o_sb = outp.tile([128, CHUNK], F32, tag="osb")
                    nc.any.tensor_copy(o_sb[:], ot_ps[ipc][:])
                    nc.sync.dma_start(
                        out=out_d[ipc, :, c * CHUNK : (c + 1) * CHUNK], in_=o_sb[:]
                    )

    nc.compile()
    return nc


def _get_nc():
    if "nc" not in _CACHE:
        _CACHE["nc"] = _build_nc()
    return _CACHE["nc"]


def _make_cmask():
    cmask = np.zeros((128, TPC * CHUNK), np.float32)
    tri = np.triu(np.ones((128, 128), np.float32))
    for pl in range(TPC):
        for qb in range(TPC):
            blk = cmask[:, pl * CHUNK + qb * 128 : pl * CHUNK + (qb + 1) * 128]
            if qb > pl:
                blk[:] = 1.0
            elif qb == pl:
                blk[:] = tri
    return cmask


def _make_in_maps(r_prime, Q, E):
    bf16 = ml_dtypes.bfloat16
    cmask = _make_cmask()
    in_maps = []
    for core in range(N_CORES):
        b, hg = core // 2, core % 2
        r = r_prime[0, b]  # [2048, 256]
        # rt[p, ic, t] = r[t, ic*128+p]
        rt = np.ascontiguousarray(
            r.T.reshape(2, 128, N_T).transpose(1, 0, 2)
        ).astype(bf16)
        # q[p, h, ic, f] = Q[h, ic*128+p, f]
        qh = np.ascontiguousarray(
            Q[0, hg * 4 : hg * 4 + 4]
            .reshape(4, 2, 128, N_IN)
            .transpose(2, 0, 1, 3)
        ).astype(bf16)
        # et[p, hp, jc, sh*256+f] = E[2hp+sh].T[jc*128+p, f]
        eth = (
            E[0, hg * 4 : hg * 4 + 4]
            .transpose(0, 2, 1)  # [h, j, i']
            .reshape(2, 2, 2, 128, N_IN)  # [hp, sh, jc, p, f]
            .transpose(3, 0, 2, 1, 4)  # [p, hp, jc, sh, f]
            .reshape(128, 2, 2, CHUNK)
        )
        eth = np.ascontiguousarray(eth).astype(bf16)
        in_maps.append({"rt": rt, "q": qh, "et": eth, "cmask": cmask})
    return in_maps


def _ensure_ntff_hook():
    """The container's `antenv` stub lacks `axon_hooks`, so the boot-time
    NTFF profile hook registration silently no-ops. Recreate it so
    trace=True yields exec_time_ns. Only used by the test harness."""
    import sys
    import types

    if "antenv.axon_hooks" not in sys.modules:
        import antenv

        mod = types.ModuleType("antenv.axon_hooks")
        state = {}
        mod.set_axon_ntff_profile_hook = lambda h: state.update(h=h)
        mod.get_axon_ntff_profile_hook = lambda: state.get("h")
        sys.modules["antenv.axon_hooks"] = mod
        antenv.axon_hooks = mod
    from antenv.axon_hooks import (
        get_axon_ntff_profile_hook,
        set_axon_ntff_profile_hook,
    )

    if get_axon_ntff_profile_hook() is None:
        from trn_agent_boot.trn_boot import _ntff_profile_via_ctypes

        set_axon_ntff_profile_hook(
            _ntff_profile_via_ctypes("/opt/axon/libaxon_pjrt.so")
        )


def _run(r_prime, Q, E, trace=False, trace_kwargs=None):
    from concourse.bass_utils import run_bass_kernel_spmd

    if trace:
        _ensure_ntff_hook()
    r_prime = np.asarray(r_prime, dtype=np.float32)
    Q = np.asarray(Q, dtype=np.float32)
    E = np.asarray(E, dtype=np.float32)
    in_maps = _make_in_maps(r_prime, Q, E)
    nc = _get_nc()
    kw = {}
    if trace:
        kw["trace"] = True
        if trace_kwargs:
            kw.update(trace_kwargs)
    res = run_bass_kernel_spmd(nc, in_maps, core_ids=list(range(N_CORES)), **kw)
    out = np.zeros((1, 4, N_T, N_IN), np.float32)
    for b in range(4):
        p0 = np.asarray(res.results[2 * b]["out"], np.float32).reshape(N_IN, N_T)
        p1 = np.asarray(res.results[2 * b + 1]["out"], np.float32).reshape(
            N_IN, N_T
        )
        out[0, b] = (p0 + p1).T
    return out, res


def kernel(r_prime, Q, E):
    out, _ = _run(r_prime, Q, E, trace=False)
    return out


# revision 13
# speedup vs baseline: 1.2085x; 1.2085x over previous
"""Trainium2 Bass kernel for bilinear causal attention (no softmax).

Math (from the reference):
  Omega[b,h,t,u] = r_t^T Q^h r_u            (scores)
  out[b,t,:]     = sum_h sum_{u<=t} Omega[b,h,t,u] * (E^h r_u)

Shapes: r_prime [1,4,2048,256] f32, Q [1,8,256,256], E [1,8,256,256],
out [1,4,2048,256] f32.

Sharding over 8 NeuronCores: core = 2*b + hg handles batch b (4 batches)
and head-group hg (heads hg*4 .. hg*4+3).  Each core produces a partial
output summed over its 4 heads; the host adds the two head-group
partials per batch.  No on-chip collectives.

Per-core algorithm (all matmuls, bf16 inputs with f32 PSUM accumulation):
  KT[h]  :  KT[i',t] = sum_i Q[i,i'] rT[i,t]            ([256,2048])
  V[h]   :  V[u,i']  = sum_j rT[j,u] ET[j,i']           ([2048,256])
  ST     :  ST[u,t]  = sum_{i'} rT[i',u] KT[i',t]       (= Omega[t,u])
  OT     :  OT[i',t] += sum_h sum_u V[u,i'] ST_masked[u,t]

All matmuls run with a 512-wide moving operand (one f32 PSUM bank).
Causality at tile granularity: u-tiles above the t-chunk are skipped;
on the chunk diagonal only the valid column range [pl*128:512] is
computed and the triangular boundary is masked elementwise.  The output
is produced transposed ([i',t]) and transposed back on the host.
"""

import numpy as np
import ml_dtypes

N_T = 2048           # sequence length t
N_IN = 256           # feature dim (i, j, i' all 256)
CHUNK = 512          # matmul free-dim chunk (one PSUM bank of f32)
NCHUNKS = N_T // CHUNK          # 4
TPC = CHUNK // 128              # 4 tiles of 128 per chunk
TQ = N_T // 128                 # 16 row tiles
N_CORES = 8

_CACHE = {}


def _build_nc():
    from concourse import mybir, bacc, tile

    BF16 = mybir.dt.bfloat16
    F32 = mybir.dt.float32

    nc = bacc.Bacc(
        "TRN2", target_bir_lowering=False, debug=False, num_devices=N_CORES
    )
    # All inputs partition-major so each loads with few large DMAs.
    rt_d = nc.dram_tensor("rt", [128, 2, N_T], BF16, kind="ExternalInput").ap()
    q_d = nc.dram_tensor("q", [128, 4, 2, N_IN], BF16, kind="ExternalInput").ap()
    # et pairs two heads side by side: [p, head-pair, j-chunk, 512]
    et_d = nc.dram_tensor("et", [128, 2, 2, CHUNK], BF16, kind="ExternalInput").ap()
    mask_d = nc.dram_tensor(
        "cmask", [128, TPC * CHUNK], F32, kind="ExternalInput"
    ).ap()
    # transposed output: [i' chunk, i' in chunk, t]
    out_d = nc.dram_tensor("out", [2, 128, N_T], F32, kind="ExternalOutput").ap()

    with tile.TileContext(nc) as tc:
        with (
            tc.tile_pool(name="consts", bufs=1) as consts,
            tc.tile_pool(name="stsb", bufs=8) as stp,
            tc.tile_pool(name="outsb", bufs=3) as outp,
            tc.tile_pool(name="psum", bufs=1, space="PSUM") as psum,
        ):
            rt_sb = consts.tile([128, 2, N_T], BF16)
            q_sb = consts.tile([128, 4, 2, N_IN], BF16)
            et_sb = consts.tile([128, 2, 2, CHUNK], BF16)
            mask_sb = consts.tile([128, TPC * CHUNK], F32)
            kt_sb = consts.tile([128, 4, 2, N_T], BF16)
            # [p, head-pair, u-tile, (head-in-pair x i')]
            v_sb = consts.tile([128, 2, TQ, 2 * N_IN], BF16)

            # Input DMAs. gpsimd's SWDGE queue measures ~170 GB/s vs the
            # HWDGE queues' ~52 GB/s, so the first-needed tensors (q head
            # 0, rt chunks, et pair 0) go through gpsimd in need-order;
            # the rest spills to sync/scalar.
            def _rt_dma(eng, tcn):
                eng.dma_start(
                    out=rt_sb[:, :, tcn * CHUNK : (tcn + 1) * CHUNK],
                    in_=rt_d[:, :, tcn * CHUNK : (tcn + 1) * CHUNK],
                )

            nc.gpsimd.dma_start(out=q_sb[:, 0], in_=q_d[:, 0])
            _rt_dma(nc.gpsimd, 0)
            _rt_dma(nc.gpsimd, 3)
            _rt_dma(nc.sync, 1)
            _rt_dma(nc.scalar, 2)
            nc.gpsimd.dma_start(out=et_sb[:, 0], in_=et_d[:, 0])
            nc.gpsimd.dma_start(out=et_sb[:, 1], in_=et_d[:, 1])
            nc.sync.dma_start(out=q_sb[:, 1], in_=q_d[:, 1])
            nc.scalar.dma_start(out=q_sb[:, 2], in_=q_d[:, 2])
            nc.sync.dma_start(out=q_sb[:, 3], in_=q_d[:, 3])
            nc.gpsimd.dma_start(out=mask_sb[:], in_=mask_d[:])

            # PE warm-up: junk matmuls on zeroed SBUF fill the DMA wait and
            # lift the HAM clock gate before real work arrives.
            junk_sb = consts.tile([128, 640], BF16)
            nc.vector.memset(junk_sb[:], 0.0)
            for _ in range(10):
                junk_ps = psum.tile([128, CHUNK], F32, tag="ps512", bufs=3)
                nc.tensor.matmul(
                    junk_ps[:], junk_sb[:, 0:128], junk_sb[:, 128:640],
                    start=True, stop=True,
                )

            # Phase A: per head, KT = (rQ)^T; per head-pair, V = r @ E^T
            # (two heads share the 512-wide moving operand). Kept in SBUF
            # as bf16. Emission order tracks DMA arrival (PE executes its
            # instruction stream in order): KT h0, KT h1, V pair0, ...
            def _kt(h, tc_order=(0, 1, 2, 3)):
                for ipc in range(2):
                    for tcn in tc_order:
                        kt_ps = psum.tile(
                            [128, CHUNK], F32, tag="ps512", bufs=3, name="kt_ps"
                        )
                        for ic in range(2):
                            nc.tensor.matmul(
                                kt_ps[:],
                                q_sb[:, h, ic, ipc * 128 : (ipc + 1) * 128],
                                rt_sb[:, ic, tcn * CHUNK : (tcn + 1) * CHUNK],
                                start=(ic == 0),
                                stop=(ic == 1),
                            )
                        nc.any.tensor_copy(
                            kt_sb[:, h, ipc, tcn * CHUNK : (tcn + 1) * CHUNK],
                            kt_ps[:],
                        )

            def _v(hp):
                for ut in range(TQ):
                    v_ps = psum.tile(
                        [128, CHUNK], F32, tag="ps512", bufs=3, name="v_ps"
                    )
                    for ic in range(2):
                        nc.tensor.matmul(
                            v_ps[:],
                            rt_sb[:, ic, ut * 128 : (ut + 1) * 128],
                            et_sb[:, hp, ic, :],
                            start=(ic == 0),
                            stop=(ic == 1),
                        )
                    nc.any.tensor_copy(v_sb[:, hp, ut, :], v_ps[:])

            _kt(0, tc_order=(0, 3, 1, 2))
            _kt(1)
            _v(0)
            _kt(2)
            _kt(3)
            _v(1)

            # Phase B: per 512-wide t-chunk, stream ST tiles and accumulate
            # the transposed output in PSUM across heads and u-tiles.
            for c in range(NCHUNKS):
                ot_ps = [
                    psum.tile(
                        [128, CHUNK], F32, tag="ot", bufs=4, name=f"ot_{c}_{ipc}"
                    )
                    for ipc in range(2)
                ]
                for ut in range(TPC * c + TPC):
                    pl = ut - TPC * c  # >= 0 on the chunk diagonal
                    off = max(pl, 0) * 128
                    for h in range(4):
                        st_ps = psum.tile([128, CHUNK], F32, tag="ps512", bufs=3)
                        for ic in range(2):
                            nc.tensor.matmul(
                                st_ps[:, off:],
                                rt_sb[:, ic, ut * 128 : (ut + 1) * 128],
                                kt_sb[:, h, ic, c * CHUNK + off : (c + 1) * CHUNK],
                                start=(ic == 0),
                                stop=(ic == 1),
                            )
                        st_sb = stp.tile([128, CHUNK], BF16, tag="st")
                        if pl >= 0:
                            nc.vector.tensor_mul(
                                st_sb[:, off:],
                                st_ps[:, off:],
                                mask_sb[:, pl * CHUNK + off : (pl + 1) * CHUNK],
                            )
                        else:
                            nc.any.tensor_copy(st_sb[:], st_ps[:])
                        for ipc in range(2):
                            voff = (h % 2) * N_IN + ipc * 128
                            nc.tensor.matmul(
                                ot_ps[ipc][:, off:],
                                v_sb[:, h // 2, ut, voff : voff + 128],
                                st_sb[:, off:],
                                start=(ut == 0 and h == 0),
                                stop=(ut == TPC * c + TPC - 1 and h == 3),
                            )
                for ipc in range(2):
                    o_sb = outp.tile([128, CHUNK], F32, tag="osb")
                    nc.any.tensor_copy(o_sb[:], ot_ps[ipc][:])
                    nc.sync.dma_start(
                        out=out_d[ipc, :, c * CHUNK : (c + 1) * CHUNK], in_=o_sb[:]
                    )

    nc.compile()
    return nc


def _get_nc():
    if "nc" not in _CACHE:
        _CACHE["nc"] = _build_nc()
    return _CACHE["nc"]


def _make_cmask():
    cmask = np.zeros((128, TPC * CHUNK), np.float32)
    tri = np.triu(np.ones((128, 128), np.float32))
    for pl in range(TPC):
        for qb in range(TPC):
            blk = cmask[:, pl * CHUNK + qb * 128 : pl * CHUNK + (qb + 1) * 128]
            if qb > pl:
                blk[:] = 1.0
            elif qb == pl:
                blk[:] = tri
    return cmask


def _make_in_maps(r_prime, Q, E):
    bf16 = ml_dtypes.bfloat16
    cmask = _make_cmask()
    in_maps = []
    for core in range(N_CORES):
        b, hg = core // 2, core % 2
        r = r_prime[0, b]  # [2048, 256]
        # rt[p, ic, t] = r[t, ic*128+p]
        rt = np.ascontiguousarray(
            r.T.reshape(2, 128, N_T).transpose(1, 0, 2)
        ).astype(bf16)
        # q[p, h, ic, f] = Q[h, ic*128+p, f]
        qh = np.ascontiguousarray(
            Q[0, hg * 4 : hg * 4 + 4]
            .reshape(4, 2, 128, N_IN)
            .transpose(2, 0, 1, 3)
        ).astype(bf16)
        # et[p, hp, jc, sh*256+f] = E[2hp+sh].T[jc*128+p, f]
        eth = (
            E[0, hg * 4 : hg * 4 + 4]
            .transpose(0, 2, 1)  # [h, j, i']
            .reshape(2, 2, 2, 128, N_IN)  # [hp, sh, jc, p, f]
            .transpose(3, 0, 2, 1, 4)  # [p, hp, jc, sh, f]
            .reshape(128, 2, 2, CHUNK)
        )
        eth = np.ascontiguousarray(eth).astype(bf16)
        in_maps.append({"rt": rt, "q": qh, "et": eth, "cmask": cmask})
    return in_maps


def _ensure_ntff_hook():
    """The container's `antenv` stub lacks `axon_hooks`, so the boot-time
    NTFF profile hook registration silently no-ops. Recreate it so
    trace=True yields exec_time_ns. Only used by the test harness."""
    import sys
    import types

    if "antenv.axon_hooks" not in sys.modules:
        import antenv

        mod = types.ModuleType("antenv.axon_hooks")
        state = {}
        mod.set_axon_ntff_profile_hook = lambda h: state.update(h=h)
        mod.get_axon_ntff_profile_hook = lambda: state.get("h")
        sys.modules["antenv.axon_hooks"] = mod
        antenv.axon_hooks = mod
    from antenv.axon_hooks import (
        get_axon_ntff_profile_hook,
        set_axon_ntff_profile_hook,
    )

    if get_axon_ntff_profile_hook() is None:
        from trn_agent_boot.trn_boot import _ntff_profile_via_ctypes

        set_axon_ntff_profile_hook(
            _ntff_profile_via_ctypes("/opt/axon/libaxon_pjrt.so")
        )


def _run(r_prime, Q, E, trace=False, trace_kwargs=None):
    from concourse.bass_utils import run_bass_kernel_spmd

    try:
        _ensure_ntff_hook()
    except Exception:
        pass  # profiling is optional; never block the actual run
    r_prime = np.asarray(r_prime, dtype=np.float32)
    Q = np.asarray(Q, dtype=np.float32)
    E = np.asarray(E, dtype=np.float32)
    in_maps = _make_in_maps(r_prime, Q, E)
    nc = _get_nc()
    kw = {}
    if trace:
        kw["trace"] = True
        if trace_kwargs:
            kw.update(trace_kwargs)
    res = run_bass_kernel_spmd(nc, in_maps, core_ids=list(range(N_CORES)), **kw)
    out = np.zeros((1, 4, N_T, N_IN), np.float32)
    for b in range(4):
        p0 = np.asarray(res.results[2 * b]["out"], np.float32).reshape(N_IN, N_T)
        p1 = np.asarray(res.results[2 * b + 1]["out"], np.float32).reshape(
            N_IN, N_T
        )
        out[0, b] = (p0 + p1).T
    return out, res


def kernel(r_prime, Q, E):
    out, _ = _run(r_prime, Q, E, trace=False)
    return out
